# revision 7
# baseline (speedup 1.0000x reference)
# Trainium2 Bass kernel for nn_ATTCNN: embedding + window-CNN (k=3,4,5) +
# span-pool + MLP head. Data-parallel over 8 NeuronCores (16 samples each).
#
# Algorithm notes (host-side weight preprocessing, exact up to fp reassociation):
#  - The conv input features concat(we[t+0], we[t+1], we[t+2], pos1v[t], pos2v[t])
#    make the conv over k taps a sum of shifted matmuls against the SAME
#    feature-major sequence. The (tap j, window m) pairs with j+m == s are
#    collapsed on the host into one weight matrix per shift s:
#        wwe_k[s] = sum_{j+m=s} w_k[:, j, 300*m:300*(m+1)]      (s in 0..k+1)
#    cutting conv MACs ~1.8x. The reference zeroes the we-window features of
#    position 0 (WF[:,0]=0); that only affects output column t=0 via tap j=0,
#    fixed by subtracting c_k[b] = sum_m we_b[m] @ w_k[:, 0, 300m:...] there.
#  - tanh is monotonic: max-then-tanh instead of tanh-then-max.
#  - Big matmuls run in bf16 (1 cy/row on PE); span/selection features (l1..l4)
#    and the MLP head stay fp32.
import os
import sys

import numpy as np

for _p in ("/opt/trn_rl_repo", "/root/.axon_site/_ro/trn_rl_repo"):
    if _p not in sys.path and os.path.isdir(_p):
        sys.path.append(_p)

import ml_dtypes  # noqa: E402

B, L, E, P, V, FN, H2, LAB = 128, 128, 300, 50, 50000, 256, 100, 19
WIN = 3
FILTERS = (3, 4, 5)
FD = 2 * P + WIN * E
NCORES = 8
BC = B // NCORES            # samples per core
LT = L + 2                  # padded token positions per sample
NTOK = BC * LT              # flat tokens per core (2080)
NTT = (NTOK + 127) // 128   # token tiles per core (17)
NPOS = BC * L               # flat pos tokens per core (2048)
NPT = NPOS // 128           # pos tiles per core (16)
ECH = (128, 128, 44)        # E=300 split over partition chunks
NSEL = 6                    # l1, l2, l3a, l3b, l4a, l4b
BF16 = ml_dtypes.bfloat16


def _prep_shared(emb, pos1, pos2, conv_w3, conv_b3, conv_w4, conv_b4,
                 conv_w5, conv_b5, W1, b1, W2, b2):
    """Host-side weight layout prep (replicated across cores)."""
    ws = {3: conv_w3[:, 0], 4: conv_w4[:, 0], 5: conv_w5[:, 0]}  # [FN, k, FD]

    # cw_we[(ki, s, ec)] = [128, FN] bf16, rows = e-chunk of collapsed wwe_k[s].
    segs = []
    for k in FILTERS:
        w = ws[k]
        wwe = np.zeros((k + 2, E, FN), np.float32)
        for j in range(k):
            for m in range(WIN):
                wwe[j + m] += w[:, j, E * m:E * (m + 1)].T
        for s in range(k + 2):
            for ec in range(3):
                seg = np.zeros((128, FN), np.float32)
                seg[:ECH[ec]] = wwe[s, 128 * ec:128 * ec + ECH[ec]]
                segs.append(seg)
    cw_we = np.stack(segs).astype(BF16)  # [54, 128, 256]

    # cw_pos[(ki, j)] = [128, FN] bf16, rows 0..99 = pos1/pos2 weights of tap j.
    segs = []
    for k in FILTERS:
        w = ws[k]
        for j in range(k):
            seg = np.zeros((128, FN), np.float32)
            seg[:2 * P] = w[:, j, WIN * E:].T
            segs.append(seg)
    cw_pos = np.stack(segs).astype(BF16)  # [12, 128, 256]

    # cw_cor[(m, ec)] = [128, 3*FN] bf16: tap-0 we weights for the t=0 fix.
    # columns kf = ki*FN + f.
    segs = []
    for m in range(WIN):
        for ec in range(3):
            seg = np.zeros((128, 3 * FN), np.float32)
            for ki, k in enumerate(FILTERS):
                seg[:ECH[ec], ki * FN:(ki + 1) * FN] = \
                    ws[k][:, 0, E * m + 128 * ec:E * m + 128 * ec + ECH[ec]].T
            segs.append(seg)
    cw_cor = np.stack(segs).astype(BF16)  # [9, 128, 768]

    # W1T segs: contraction over sf features (3*FN=768) in 6 chunks of 128.
    w1t = np.ascontiguousarray(W1.T.reshape(6, 128, H2).astype(np.float32))

    # W2T segs: 18 = (piece, ec) over l1,l2,l3a,l3b,l4a,l4b + 1 for g.
    segs = []
    for p in range(NSEL):
        for ec in range(3):
            seg = np.zeros((128, LAB), np.float32)
            seg[:ECH[ec]] = W2[:, E * p + 128 * ec:E * p + 128 * ec + ECH[ec]].T
            segs.append(seg)
    gseg = np.zeros((128, LAB), np.float32)
    gseg[:H2] = W2[:, NSEL * E:].T
    segs.append(gseg)
    w2t = np.stack(segs)  # [19, 128, 19]

    # conv biases per (ki, mt) as bias columns.
    cb = np.zeros((128, 6), np.float32)
    for ki, k in enumerate(FILTERS):
        bk = {3: conv_b3, 4: conv_b4, 5: conv_b5}[k]
        cb[:, 2 * ki] = bk[:128]
        cb[:, 2 * ki + 1] = bk[128:]
    b1p = np.zeros((128, 1), np.float32)
    b1p[:H2, 0] = b1
    b2p = np.asarray(b2, np.float32).reshape(LAB, 1)

    return dict(
        emb=np.asarray(emb, np.float32),
        pos1=np.asarray(pos1, np.float32), pos2=np.asarray(pos2, np.float32),
        cw_we=cw_we, cw_pos=cw_pos, cw_cor=cw_cor,
        w1t=w1t, w2t=w2t, cb=cb, b1=b1p, b2=b2p,
    )


def _prep_core(c, inputs, e1s, e1e, e2s, e2e, p1, p2):
    """Host-side per-core index/mask prep."""
    sl = slice(c * BC, (c + 1) * BC)
    inp = np.asarray(inputs[sl], np.int64)
    tok = np.zeros((BC, LT), np.int32)
    tok[:, 1:1 + L] = inp
    tok_flat = np.zeros(NTT * 128, np.int32)
    tok_flat[:NTOK] = tok.reshape(-1)
    tok_idx = np.ascontiguousarray(tok_flat.reshape(NTT, 128).T)  # [128, 17]

    p1i = np.ascontiguousarray(
        np.asarray(p1[sl], np.int64).astype(np.int32).reshape(NPT, 128).T)
    p2i = np.ascontiguousarray(
        np.asarray(p2[sl], np.int64).astype(np.int32).reshape(NPT, 128).T)

    # bigmask [17, 128, 96]: col = lb*6 + sel over flat token q = lb*130 + i.
    bigmask = np.zeros((NTT * 128, BC * NSEL), np.float32)
    for lb in range(BC):
        b = c * BC + lb
        s1, t1 = int(e1s[b]), int(e1e[b])
        s2, t2 = int(e2s[b]), int(e2e[b])
        q0 = lb * LT
        bigmask[q0 + s1 + 1:q0 + t1 + 2, lb * NSEL + 0] = 1.0 / (t1 - s1 + 1)
        bigmask[q0 + s2 + 1:q0 + t2 + 2, lb * NSEL + 1] = 1.0 / (t2 - s2 + 1)
        bigmask[q0 + s1, lb * NSEL + 2] = 1.0
        bigmask[q0 + t1 + 2, lb * NSEL + 3] = 1.0
        bigmask[q0 + s2, lb * NSEL + 4] = 1.0
        bigmask[q0 + t2 + 2, lb * NSEL + 5] = 1.0
    bigmask = np.ascontiguousarray(bigmask.reshape(NTT, 128, BC * NSEL))

    return dict(tok_idx=tok_idx, p1_idx=p1i, p2_idx=p2i, bigmask=bigmask)


def _build_nc():
    import concourse.bacc as bacc
    import concourse.tile as tile
    from concourse import mybir
    from concourse.bass import IndirectOffsetOnAxis
    from concourse.masks import make_identity

    f32, bf16, i32 = mybir.dt.float32, mybir.dt.bfloat16, mybir.dt.int32

    nc = bacc.Bacc("TRN2", target_bir_lowering=False, debug=False,
                   num_devices=NCORES)

    # ---- DRAM I/O ----
    tok_idx_d = nc.dram_tensor("tok_idx", [128, NTT], i32, kind="ExternalInput")
    p1_idx_d = nc.dram_tensor("p1_idx", [128, NPT], i32, kind="ExternalInput")
    p2_idx_d = nc.dram_tensor("p2_idx", [128, NPT], i32, kind="ExternalInput")
    emb_d = nc.dram_tensor("emb", [V, E], f32, kind="ExternalInput")
    pos1_d = nc.dram_tensor("pos1", [2 * L + 3, P], f32, kind="ExternalInput")
    pos2_d = nc.dram_tensor("pos2", [2 * L + 3, P], f32, kind="ExternalInput")
    cw_we_d = nc.dram_tensor("cw_we", [54, 128, FN], bf16, kind="ExternalInput")
    cw_pos_d = nc.dram_tensor("cw_pos", [12, 128, FN], bf16, kind="ExternalInput")
    cw_cor_d = nc.dram_tensor("cw_cor", [9, 128, 3 * FN], bf16, kind="ExternalInput")
    w1t_d = nc.dram_tensor("w1t", [6, 128, H2], f32, kind="ExternalInput")
    w2t_d = nc.dram_tensor("w2t", [19, 128, LAB], f32, kind="ExternalInput")
    cb_d = nc.dram_tensor("cb", [128, 6], f32, kind="ExternalInput")
    b1_d = nc.dram_tensor("b1", [128, 1], f32, kind="ExternalInput")
    b2_d = nc.dram_tensor("b2", [LAB, 1], f32, kind="ExternalInput")
    bigmask_d = nc.dram_tensor("bigmask", [NTT, 128, BC * NSEL], f32,
                               kind="ExternalInput")
    y_d = nc.dram_tensor("y", [LAB, BC], f32, kind="ExternalOutput")

    with tile.TileContext(nc) as tc:
        # PSUM budget (8 banks): tp 2 + lps 3 + zp 2 + aux 1.
        with tc.tile_pool(name="persist", bufs=1) as pp, \
             tc.tile_pool(name="work", bufs=3) as wp, \
             tc.tile_pool(name="tp_ps", bufs=2, space="PSUM") as tp_ps, \
             tc.tile_pool(name="l_ps", bufs=1, space="PSUM") as l_ps, \
             tc.tile_pool(name="z_ps", bufs=2, space="PSUM") as z_ps, \
             tc.tile_pool(name="aux_ps", bufs=1, space="PSUM") as aux_ps:

            ident = pp.tile([128, 128], f32)
            make_identity(nc, ident[:])

            # ---- persistent SBUF ----
            weT = [pp.tile([128, NTT * 128], bf16, tag=f"weT{ec}", name=f"weT{ec}")
                   for ec in range(3)]
            posT = pp.tile([128, NPOS], bf16)
            cw_we_sb = pp.tile([128, 54 * FN], bf16)
            cw_pos_sb = pp.tile([128, 12 * FN], bf16)
            cw_cor_sb = pp.tile([128, 9 * 3 * FN], bf16)
            w1t_sb = pp.tile([128, 6 * H2], f32)
            w2t_sb = pp.tile([128, 19 * LAB], f32)
            cb_sb = pp.tile([128, 6], f32)
            b1_sb = pp.tile([128, 1], f32)
            b2_sb = pp.tile([LAB, 1], f32)
            mask_sb = pp.tile([128, NTT * BC * NSEL], f32)
            tok_idx_sb = pp.tile([128, NTT], i32)
            p1_idx_sb = pp.tile([128, NPT], i32)
            p2_idx_sb = pp.tile([128, NPT], i32)
            cor_sb = pp.tile([128, 6 * BC], f32)
            lvec = [pp.tile([128, BC * NSEL], f32, tag=f"lvec{ec}", name=f"lvec{ec}")
                    for ec in range(3)]
            sf = [pp.tile([128, BC], f32, tag=f"sf{i}", name=f"sf{i}")
                  for i in range(6)]
            g_sb = pp.tile([128, BC], f32)
            y_sb = pp.tile([LAB, BC], f32)

            # ---- input loads ----
            nc.sync.dma_start(tok_idx_sb[:], tok_idx_d[:])
            nc.sync.dma_start(p1_idx_sb[:], p1_idx_d[:])
            nc.sync.dma_start(p2_idx_sb[:], p2_idx_d[:])
            nc.sync.dma_start(
                cw_we_sb[:].rearrange("p (s f) -> p s f", s=54),
                cw_we_d[:].rearrange("s p f -> p s f"))
            nc.sync.dma_start(
                cw_pos_sb[:].rearrange("p (s f) -> p s f", s=12),
                cw_pos_d[:].rearrange("s p f -> p s f"))
            nc.sync.dma_start(
                cw_cor_sb[:].rearrange("p (s f) -> p s f", s=9),
                cw_cor_d[:].rearrange("s p f -> p s f"))
            nc.sync.dma_start(
                w1t_sb[:].rearrange("p (s f) -> p s f", s=6),
                w1t_d[:].rearrange("s p f -> p s f"))
            nc.sync.dma_start(
                w2t_sb[:].rearrange("p (s f) -> p s f", s=19),
                w2t_d[:].rearrange("s p f -> p s f"))
            nc.sync.dma_start(cb_sb[:], cb_d[:])
            nc.sync.dma_start(b1_sb[:], b1_d[:])
            nc.sync.dma_start(b2_sb[:], b2_d[:])
            nc.sync.dma_start(
                mask_sb[:].rearrange("p (s f) -> p s f", s=NTT),
                bigmask_d[:].rearrange("s p f -> p s f"))

            # ---- phase B: embedding gather, l-matmuls, transposes ----
            lps = [l_ps.tile([128, BC * NSEL], f32, tag=f"lps{ec}", name=f"lps{ec}")
                   for ec in range(3)]
            for t in range(NTT):
                tokt = wp.tile([128, E], f32, tag="tokt")
                nc.gpsimd.indirect_dma_start(
                    out=tokt[:], out_offset=None, in_=emb_d[:],
                    in_offset=IndirectOffsetOnAxis(
                        ap=tok_idx_sb[:, t:t + 1], axis=0))
                for ec in range(3):
                    nc.tensor.matmul(
                        lps[ec][:ECH[ec], :],
                        tokt[:, 128 * ec:128 * ec + ECH[ec]],
                        mask_sb[:, t * BC * NSEL:(t + 1) * BC * NSEL],
                        start=(t == 0), stop=(t == NTT - 1))
                    tp = tp_ps.tile([128, 128], f32, tag="tp")
                    nc.tensor.transpose(
                        out=tp[:ECH[ec], :],
                        in_=tokt[:, 128 * ec:128 * ec + ECH[ec]],
                        identity=ident[:])
                    nc.vector.tensor_copy(
                        weT[ec][:ECH[ec], t * 128:(t + 1) * 128],
                        tp[:ECH[ec], :])
            for ec in range(3):
                nc.vector.tensor_copy(lvec[ec][:ECH[ec], :], lps[ec][:ECH[ec], :])

            # ---- phase C: pos gathers + transposes ----
            for t in range(NPT):
                post = wp.tile([128, 2 * P], f32, tag="post")
                nc.gpsimd.indirect_dma_start(
                    out=post[:, :P], out_offset=None, in_=pos1_d[:],
                    in_offset=IndirectOffsetOnAxis(
                        ap=p1_idx_sb[:, t:t + 1], axis=0))
                nc.gpsimd.indirect_dma_start(
                    out=post[:, P:], out_offset=None, in_=pos2_d[:],
                    in_offset=IndirectOffsetOnAxis(
                        ap=p2_idx_sb[:, t:t + 1], axis=0))
                tp = tp_ps.tile([128, 128], f32, tag="tp")
                nc.tensor.transpose(out=tp[:2 * P, :], in_=post[:],
                                    identity=ident[:])
                nc.vector.tensor_copy(posT[:2 * P, t * 128:(t + 1) * 128],
                                      tp[:2 * P, :])

            # ---- phase D: t=0 correction, c[(ki,f), b] ----
            weT3 = [weT[ec][:, :NTOK].rearrange("p (b t) -> p b t", t=LT)
                    for ec in range(3)]
            for mt in range(6):
                corp = aux_ps.tile([128, BC], f32, tag="aux")
                corp3 = corp[:].rearrange("p (b o) -> p b o", o=1)
                n = 0
                for m in range(WIN):
                    for ec in range(3):
                        nc.tensor.matmul(
                            corp3[:, :, :],
                            cw_cor_sb[:ECH[ec],
                                      (3 * m + ec) * 3 * FN + mt * 128:
                                      (3 * m + ec) * 3 * FN + (mt + 1) * 128],
                            weT3[ec][:ECH[ec], :, m:m + 1],
                            start=(n == 0), stop=(n == 8))
                        n += 1
                nc.vector.tensor_copy(cor_sb[:, mt * BC:(mt + 1) * BC], corp[:])

            # ---- phase E: conv z, correction, max ----
            posT3 = posT[:2 * P, :].rearrange("p (b t) -> p b t", t=L)
            for ki, k in enumerate(FILTERS):
                nz = L - k + 1
                for mt in range(2):
                    for grp in range(BC // 4):
                        zp = z_ps.tile([128, 4 * nz], f32, tag="zp")
                        zp3 = zp[:].rearrange("p (b t) -> p b t", b=4)
                        n = 0
                        nmm = 3 * (k + 2) + k
                        for s in range(k + 2):
                            for ec in range(3):
                                seg = sum(kk + 2 for kk in FILTERS[:ki]) * 3 \
                                    + s * 3 + ec
                                nc.tensor.matmul(
                                    zp3[:, :, :],
                                    cw_we_sb[:ECH[ec],
                                             seg * FN + mt * 128:
                                             seg * FN + mt * 128 + 128],
                                    weT3[ec][:ECH[ec], 4 * grp:4 * grp + 4,
                                             s:s + nz],
                                    start=(n == 0), stop=(n == nmm - 1))
                                n += 1
                        for j in range(k):
                            seg = sum(FILTERS[:ki]) + j
                            nc.tensor.matmul(
                                zp3[:, :, :],
                                cw_pos_sb[:2 * P,
                                          seg * FN + mt * 128:
                                          seg * FN + mt * 128 + 128],
                                posT3[:, 4 * grp:4 * grp + 4, j:j + nz],
                                start=(n == 0), stop=(n == nmm - 1))
                            n += 1
                        # subtract t=0 correction for these 4 samples
                        nc.vector.tensor_sub(
                            zp3[:, :, 0:1],
                            zp3[:, :, 0:1],
                            cor_sb[:, (2 * ki + mt) * BC + 4 * grp:
                                   (2 * ki + mt) * BC + 4 * grp + 4]
                            .rearrange("p (b o) -> p b o", o=1))
                        nc.vector.reduce_max(
                            sf[2 * ki + mt][:, 4 * grp:4 * grp + 4],
                            zp3[:, :, :], axis=mybir.AxisListType.X)

            # ---- phase F: sf tanh, g, y ----
            sft = [pp.tile([128, BC], f32, tag=f"sft{i}", name=f"sft{i}")
                   for i in range(6)]
            gp = aux_ps.tile([128, BC], f32, tag="aux")
            for i in range(6):
                nc.scalar.activation(sft[i][:], sf[i][:],
                                     mybir.ActivationFunctionType.Tanh,
                                     bias=cb_sb[:, i:i + 1])
                nc.tensor.matmul(gp[:H2, :], w1t_sb[:, i * H2:(i + 1) * H2],
                                 sft[i][:], start=(i == 0), stop=(i == 5))
            nc.scalar.activation(g_sb[:H2, :], gp[:H2, :],
                                 mybir.ActivationFunctionType.Tanh,
                                 bias=b1_sb[:H2, :1])

            yp = aux_ps.tile([LAB, BC], f32, tag="aux")
            yp3 = yp[:].rearrange("p (o b) -> p o b", o=1)
            n = 0
            for p in range(NSEL):
                for ec in range(3):
                    nc.tensor.matmul(
                        yp3[:, :, :],
                        w2t_sb[:ECH[ec], (3 * p + ec) * LAB:
                               (3 * p + ec + 1) * LAB],
                        lvec[ec][:ECH[ec], :].rearrange(
                            "p (b s) -> p s b", s=NSEL)[:, p:p + 1, :],
                        start=(n == 0), stop=False)
                    n += 1
            nc.tensor.matmul(yp[:], w2t_sb[:H2, 18 * LAB:19 * LAB],
                             g_sb[:H2, :], start=False, stop=True)
            nc.scalar.activation(y_sb[:], yp[:],
                                 mybir.ActivationFunctionType.Identity,
                                 bias=b2_sb[:, :1])
            nc.sync.dma_start(y_d[:], y_sb[:])

    nc.compile()
    return nc


_NC = None
_LAST = None


def kernel(inputs, e1s, e1e, e2s, e2e, p1, p2, emb, pos1, pos2,
           conv_w3, conv_b3, conv_w4, conv_b4, conv_w5, conv_b5,
           W1, b1, W2, b2):
    global _NC
    from concourse.bass_utils import run_bass_kernel_spmd

    shared = _prep_shared(emb, pos1, pos2, conv_w3, conv_b3, conv_w4, conv_b4,
                          conv_w5, conv_b5, W1, b1, W2, b2)
    in_maps = []
    for c in range(NCORES):
        m = dict(shared)
        m.update(_prep_core(c, inputs, e1s, e1e, e2s, e2e, p1, p2))
        in_maps.append(m)

    if _NC is None:
        _NC = _build_nc()

    trace = bool(int(os.environ.get("ATTCNN_TRACE", "0")))
    res = run_bass_kernel_spmd(_NC, in_maps, core_ids=list(range(NCORES)),
                               trace=trace)
    global _LAST
    _LAST = res
    y = np.zeros((B, LAB), np.float32)
    for c in range(NCORES):
        y[c * BC:(c + 1) * BC] = res.results[c]["y"].T
    return y


# revision 14
# speedup vs baseline: 1.0741x; 1.0741x over previous
# Trainium2 Bass kernel for nn_ATTCNN: embedding + window-CNN (k=3,4,5) +
# span-pool + MLP head. Data-parallel over 8 NeuronCores (16 samples each).
#
# Algorithm notes (host-side weight preprocessing, exact up to fp reassociation):
#  - The conv input features concat(we[t+0], we[t+1], we[t+2], pos1v[t], pos2v[t])
#    make the conv over k taps a sum of shifted matmuls against the SAME
#    feature-major sequence. The (tap j, window m) pairs with j+m == s are
#    collapsed on the host into one weight matrix per shift s:
#        wwe_k[s] = sum_{j+m=s} w_k[:, j, 300*m:300*(m+1)]      (s in 0..k+1)
#    cutting conv MACs ~1.8x. The reference zeroes the we-window features of
#    position 0 (WF[:,0]=0); that only affects output column t=0 via tap j=0,
#    fixed by subtracting c_k[b] = sum_m we_b[m] @ w_k[:, 0, 300m:...] there.
#  - tanh is monotonic: max-then-tanh instead of tanh-then-max.
#  - Big matmuls run in bf16 (1 cy/row on PE); span/selection features (l1..l4)
#    and the MLP head stay fp32.
import os
import sys

import numpy as np

for _p in ("/opt/trn_rl_repo", "/root/.axon_site/_ro/trn_rl_repo"):
    if _p not in sys.path and os.path.isdir(_p):
        sys.path.append(_p)

import ml_dtypes  # noqa: E402

B, L, E, P, V, FN, H2, LAB = 128, 128, 300, 50, 50000, 256, 100, 19
WIN = 3
FILTERS = (3, 4, 5)
FD = 2 * P + WIN * E
NCORES = 8
BC = B // NCORES            # samples per core
LT = L + 2                  # padded token positions per sample
NTOK = BC * LT              # flat tokens per core (2080)
NTT = (NTOK + 127) // 128   # token tiles per core (17)
NPOS = BC * L               # flat pos tokens per core (2048)
NPT = NPOS // 128           # pos tiles per core (16)
ECH = (128, 128, 44)        # E=300 split over partition chunks
NSEL = 6                    # l1, l2, l3a, l3b, l4a, l4b
BF16 = ml_dtypes.bfloat16


def _prep_shared(emb, pos1, pos2, conv_w3, conv_b3, conv_w4, conv_b4,
                 conv_w5, conv_b5, W1, b1, W2, b2):
    """Host-side weight layout prep (replicated across cores)."""
    ws = {3: conv_w3[:, 0], 4: conv_w4[:, 0], 5: conv_w5[:, 0]}  # [FN, k, FD]

    # cw_we[(ki, s, ec)] = [128, FN] bf16, rows = e-chunk of collapsed wwe_k[s].
    segs = []
    for k in FILTERS:
        w = ws[k]
        wwe = np.zeros((k + 2, E, FN), np.float32)
        for j in range(k):
            for m in range(WIN):
                wwe[j + m] += w[:, j, E * m:E * (m + 1)].T
        for s in range(k + 2):
            for ec in range(3):
                seg = np.zeros((128, FN), np.float32)
                seg[:ECH[ec]] = wwe[s, 128 * ec:128 * ec + ECH[ec]]
                segs.append(seg)
    cw_we = np.stack(segs).astype(BF16)  # [54, 128, 256]
    off = np.cumsum([0] + [3 * (k + 2) for k in FILTERS])
    cw_we3 = [np.ascontiguousarray(
        cw_we[off[ki]:off[ki + 1]].transpose(1, 0, 2)
        .reshape(128, 3 * (k + 2) * FN))
        for ki, k in enumerate(FILTERS)]

    # cw_pos[(ki, j)] = [128, FN] bf16, rows 0..99 = pos1/pos2 weights of tap j.
    segs = []
    for k in FILTERS:
        w = ws[k]
        for j in range(k):
            seg = np.zeros((128, FN), np.float32)
            seg[:2 * P] = w[:, j, WIN * E:].T
            segs.append(seg)
    cw_pos = np.ascontiguousarray(
        np.stack(segs).astype(BF16).transpose(1, 0, 2).reshape(128, 12 * FN))

    # cw_cor[(m, ec)] = [128, 3*FN] bf16: tap-0 we weights for the t=0 fix.
    # columns kf = ki*FN + f.
    segs = []
    for m in range(WIN):
        for ec in range(3):
            seg = np.zeros((128, 3 * FN), np.float32)
            for ki, k in enumerate(FILTERS):
                seg[:ECH[ec], ki * FN:(ki + 1) * FN] = \
                    ws[k][:, 0, E * m + 128 * ec:E * m + 128 * ec + ECH[ec]].T
            segs.append(seg)
    cw_cor = np.ascontiguousarray(
        np.stack(segs).astype(BF16).transpose(1, 0, 2).reshape(128, 9 * 3 * FN))

    # poscomb [5, 128, 100] bf16: stacked pos tables over a 518-row combined
    # one-hot index space (rows 0..258 pos1 -> cols 0..49, 259..517 pos2).
    M = np.zeros((5 * 128, 2 * P), np.float32)
    M[:2 * L + 3, :P] = pos1
    M[2 * L + 3:2 * (2 * L + 3), P:] = pos2
    poscomb = np.ascontiguousarray(
        M.reshape(5, 128, 2 * P).transpose(1, 0, 2).reshape(128, 5 * 2 * P)
    ).astype(BF16)

    # W1T segs: contraction over sf features (3*FN=768) in 6 chunks of 128.
    w1t = np.ascontiguousarray(
        W1.T.reshape(6, 128, H2).transpose(1, 0, 2).reshape(128, 6 * H2)
    ).astype(np.float32)

    # W2T segs: 18 = (piece, ec) over l1,l2,l3a,l3b,l4a,l4b + 1 for g.
    segs = []
    for p in range(NSEL):
        for ec in range(3):
            seg = np.zeros((128, LAB), np.float32)
            seg[:ECH[ec]] = W2[:, E * p + 128 * ec:E * p + 128 * ec + ECH[ec]].T
            segs.append(seg)
    gseg = np.zeros((128, LAB), np.float32)
    gseg[:H2] = W2[:, NSEL * E:].T
    segs.append(gseg)
    w2t = np.ascontiguousarray(
        np.stack(segs).transpose(1, 0, 2).reshape(128, 19 * LAB)
    ).astype(np.float32)

    # conv biases per (ki, mt) as bias columns.
    cb = np.zeros((128, 6), np.float32)
    for ki, k in enumerate(FILTERS):
        bk = {3: conv_b3, 4: conv_b4, 5: conv_b5}[k]
        cb[:, 2 * ki] = bk[:128]
        cb[:, 2 * ki + 1] = bk[128:]
    b1p = np.zeros((128, 1), np.float32)
    b1p[:H2, 0] = b1
    b2p = np.asarray(b2, np.float32).reshape(LAB, 1)

    return dict(
        emb=np.asarray(emb, np.float32), poscomb=poscomb,
        cw_we3=cw_we3[0], cw_we4=cw_we3[1], cw_we5=cw_we3[2],
        cw_pos=cw_pos, cw_cor=cw_cor,
        w1t=w1t, w2t=w2t, cb=cb, b1=b1p, b2=b2p,
    )


def _prep_core(c, inputs, e1s, e1e, e2s, e2e, p1, p2):
    """Host-side per-core index/mask prep."""
    sl = slice(c * BC, (c + 1) * BC)
    inp = np.asarray(inputs[sl], np.int64)
    tok = np.zeros((BC, LT), np.int32)
    tok[:, 1:1 + L] = inp
    tok_flat = np.zeros(NTT * 128, np.int32)
    tok_flat[:NTOK] = tok.reshape(-1)
    tok_idx = np.ascontiguousarray(tok_flat.reshape(NTT, 128).T)  # [128, 17]

    # onehcomb [5, 128, 2048] bf16: combined one-hot over the 518-row space;
    # each column q = lb*128 + t has two ones: p1[q] and 259 + p2[q].
    p1f = np.asarray(p1[sl], np.int64).reshape(-1)
    p2f = np.asarray(p2[sl], np.int64).reshape(-1)
    oneh = np.zeros((5 * 128, NPOS), np.float32)
    q = np.arange(NPOS)
    oneh[p1f, q] = 1.0
    oneh[(2 * L + 3) + p2f, q] = 1.0
    onehcomb = np.ascontiguousarray(
        oneh.reshape(5, 128, NPOS).transpose(1, 0, 2).reshape(128, 5 * NPOS)
    ).astype(BF16)

    # bigmask [17, 128, 96]: col = lb*6 + sel over flat token q = lb*130 + i.
    bigmask = np.zeros((NTT * 128, BC * NSEL), np.float32)
    for lb in range(BC):
        b = c * BC + lb
        s1, t1 = int(e1s[b]), int(e1e[b])
        s2, t2 = int(e2s[b]), int(e2e[b])
        q0 = lb * LT
        bigmask[q0 + s1 + 1:q0 + t1 + 2, lb * NSEL + 0] = 1.0 / (t1 - s1 + 1)
        bigmask[q0 + s2 + 1:q0 + t2 + 2, lb * NSEL + 1] = 1.0 / (t2 - s2 + 1)
        bigmask[q0 + s1, lb * NSEL + 2] = 1.0
        bigmask[q0 + t1 + 2, lb * NSEL + 3] = 1.0
        bigmask[q0 + s2, lb * NSEL + 4] = 1.0
        bigmask[q0 + t2 + 2, lb * NSEL + 5] = 1.0
    bigmask = np.ascontiguousarray(
        bigmask.reshape(NTT, 128, BC * NSEL).transpose(1, 0, 2)
        .reshape(128, NTT * BC * NSEL))

    return dict(tok_idx=tok_idx, onehcomb=onehcomb, bigmask=bigmask)


def _build_nc(ablate=()):
    import concourse.bacc as bacc
    import concourse.tile as tile
    from concourse import mybir
    from concourse.bass import IndirectOffsetOnAxis
    from concourse.masks import make_identity

    f32, bf16, i32 = mybir.dt.float32, mybir.dt.bfloat16, mybir.dt.int32

    nc = bacc.Bacc("TRN2", target_bir_lowering=False, debug=False,
                   num_devices=NCORES)

    # ---- DRAM I/O ----
    tok_idx_d = nc.dram_tensor("tok_idx", [128, NTT], i32, kind="ExternalInput")
    emb_d = nc.dram_tensor("emb", [V, E], f32, kind="ExternalInput")
    poscomb_d = nc.dram_tensor("poscomb", [128, 5 * 2 * P], bf16,
                               kind="ExternalInput")
    onehcomb_d = nc.dram_tensor("onehcomb", [128, 5 * NPOS], bf16,
                                kind="ExternalInput")
    cw_we_d = [nc.dram_tensor(f"cw_we{k}", [128, 3 * (k + 2) * FN], bf16,
                              kind="ExternalInput") for k in FILTERS]
    cw_pos_d = nc.dram_tensor("cw_pos", [128, 12 * FN], bf16,
                              kind="ExternalInput")
    cw_cor_d = nc.dram_tensor("cw_cor", [128, 9 * 3 * FN], bf16,
                              kind="ExternalInput")
    w1t_d = nc.dram_tensor("w1t", [128, 6 * H2], f32, kind="ExternalInput")
    w2t_d = nc.dram_tensor("w2t", [128, 19 * LAB], f32, kind="ExternalInput")
    cb_d = nc.dram_tensor("cb", [128, 6], f32, kind="ExternalInput")
    b1_d = nc.dram_tensor("b1", [128, 1], f32, kind="ExternalInput")
    b2_d = nc.dram_tensor("b2", [LAB, 1], f32, kind="ExternalInput")
    bigmask_d = nc.dram_tensor("bigmask", [128, NTT * BC * NSEL], f32,
                               kind="ExternalInput")
    y_d = nc.dram_tensor("y", [LAB, BC], f32, kind="ExternalOutput")

    with tile.TileContext(nc) as tc:
        # PSUM budget (8 banks): shared zp tag 4 + lps 3 + aux 1.
        with tc.tile_pool(name="persist", bufs=1) as pp, \
             tc.tile_pool(name="work", bufs=3) as wp, \
             tc.tile_pool(name="l_ps", bufs=1, space="PSUM") as l_ps, \
             tc.tile_pool(name="z_ps", bufs=4, space="PSUM") as z_ps, \
             tc.tile_pool(name="aux_ps", bufs=1, space="PSUM") as aux_ps:
            tp_ps = z_ps

            ident = pp.tile([128, 128], f32)
            make_identity(nc, ident[:])

            # ---- persistent SBUF ----
            weT = [pp.tile([128, NTT * 128], bf16, tag=f"weT{ec}", name=f"weT{ec}")
                   for ec in range(3)]
            posT = pp.tile([128, NPOS], bf16)
            cw_we_sb = [pp.tile([128, 3 * (k + 2) * FN], bf16, tag=f"cwwe{k}",
                                name=f"cwwe{k}") for k in FILTERS]
            cw_pos_sb = pp.tile([128, 12 * FN], bf16)
            cw_cor_sb = pp.tile([128, 9 * 3 * FN], bf16)
            w1t_sb = pp.tile([128, 6 * H2], f32)
            w2t_sb = pp.tile([128, 19 * LAB], f32)
            cb_sb = pp.tile([128, 6], f32)
            b1_sb = pp.tile([128, 1], f32)
            b2_sb = pp.tile([LAB, 1], f32)
            mask_sb = pp.tile([128, NTT * BC * NSEL], f32)
            tok_idx_sb = pp.tile([128, NTT], i32)
            poscomb_sb = pp.tile([128, 5 * 2 * P], bf16)
            onehcomb_sb = pp.tile([128, 5 * NPOS], bf16)
            cor_sb = pp.tile([128, 6 * BC], f32)
            lvec = [pp.tile([128, BC * NSEL], f32, tag=f"lvec{ec}", name=f"lvec{ec}")
                    for ec in range(3)]
            sf = [pp.tile([128, BC], f32, tag=f"sf{i}", name=f"sf{i}")
                  for i in range(6)]
            g_sb = pp.tile([128, BC], f32)
            y_sb = pp.tile([LAB, BC], f32)

            # ---- early loads (phase B/C deps) on the HWDGE path ----
            nc.sync.dma_start(tok_idx_sb[:], tok_idx_d[:])
            nc.sync.dma_start(mask_sb[:], bigmask_d[:])
            nc.sync.dma_start(poscomb_sb[:], poscomb_d[:])
            nc.sync.dma_start(onehcomb_sb[:], onehcomb_d[:])

            # ---- phase B: embedding gather, l-matmuls, transposes ----
            lps = [l_ps.tile([128, BC * NSEL], f32, tag=f"lps{ec}", name=f"lps{ec}")
                   for ec in range(3)]
            for t in range(NTT):
                tokt = wp.tile([128, E], f32, tag="tokt")
                if "gather" not in ablate:
                    nc.gpsimd.indirect_dma_start(
                        out=tokt[:], out_offset=None, in_=emb_d[:],
                        in_offset=IndirectOffsetOnAxis(
                            ap=tok_idx_sb[:, t:t + 1], axis=0))
                else:
                    nc.scalar.memzero(tokt[:])
                if "masks" in ablate and t == 0:
                    for ec in range(3):
                        nc.vector.memset(lps[ec][:], 0.0)
                for ec in range(3):
                    tp = z_ps.tile([128, 512], f32, tag="zp", name="tp")
                    nc.tensor.transpose(
                        out=tp[:ECH[ec], :128],
                        in_=tokt[:, 128 * ec:128 * ec + ECH[ec]],
                        identity=ident[:])
                    nc.vector.tensor_copy(
                        weT[ec][:ECH[ec], t * 128:(t + 1) * 128],
                        tp[:ECH[ec], :128])
                for ec in range(3):
                    if "masks" not in ablate:
                        nc.tensor.matmul(
                            lps[ec][:ECH[ec], :],
                            tokt[:, 128 * ec:128 * ec + ECH[ec]],
                            mask_sb[:, t * BC * NSEL:(t + 1) * BC * NSEL],
                            start=(t == 0), stop=(t == NTT - 1))
            for ec in range(3):
                nc.vector.tensor_copy(lvec[ec][:ECH[ec], :], lps[ec][:ECH[ec], :])

            # ---- weight loads (SWDGE; queue behind the emb gathers) ----
            nc.gpsimd.dma_start(cw_cor_sb[:], cw_cor_d[:])
            for ki in range(3):
                nc.gpsimd.dma_start(cw_we_sb[ki][:], cw_we_d[ki][:])
            nc.gpsimd.dma_start(cw_pos_sb[:], cw_pos_d[:])
            nc.gpsimd.dma_start(w1t_sb[:], w1t_d[:])
            nc.gpsimd.dma_start(w2t_sb[:], w2t_d[:])
            nc.gpsimd.dma_start(cb_sb[:], cb_d[:])
            nc.gpsimd.dma_start(b1_sb[:], b1_d[:])
            nc.gpsimd.dma_start(b2_sb[:], b2_d[:])

            # ---- phase C: posT via combined one-hot matmuls ----
            VCH = (128, 128, 128, 128, 6)
            for g4 in range(NPOS // 512):
                ptp = z_ps.tile([128, 512], f32, tag="zp", name="ptp")
                for vc in range(5):
                    nc.tensor.matmul(
                        ptp[:2 * P, :],
                        poscomb_sb[:VCH[vc], vc * 2 * P:(vc + 1) * 2 * P],
                        onehcomb_sb[:VCH[vc],
                                    vc * NPOS + g4 * 512:vc * NPOS + (g4 + 1) * 512],
                        start=(vc == 0), stop=(vc == 4))
                nc.vector.tensor_copy(posT[:2 * P, g4 * 512:(g4 + 1) * 512],
                                      ptp[:2 * P, :])

            # ---- phase D: t=0 correction, c[(ki,f), b] ----
            weT3 = [weT[ec][:, :NTOK].rearrange("p (b t) -> p b t", t=LT)
                    for ec in range(3)]
            for mt in range(6):
                corp = aux_ps.tile([128, BC], f32, tag="aux")
                corp3 = corp[:].rearrange("p (b o) -> p b o", o=1)
                n = 0
                for m in range(WIN):
                    for ec in range(3):
                        nc.tensor.matmul(
                            corp3[:, :, :],
                            cw_cor_sb[:ECH[ec],
                                      (3 * m + ec) * 3 * FN + mt * 128:
                                      (3 * m + ec) * 3 * FN + (mt + 1) * 128],
                            weT3[ec][:ECH[ec], :, m:m + 1],
                            start=(n == 0), stop=(n == 8))
                        n += 1
                nc.vector.tensor_copy(cor_sb[:, mt * BC:(mt + 1) * BC], corp[:])

            # ---- phase E: conv z, correction, max ----
            posT3 = posT[:2 * P, :].rearrange("p (b t) -> p b t", t=L)
            for ki, k in enumerate(FILTERS):
                if "conv" in ablate:
                    for i in range(6):
                        nc.vector.memset(sf[i][:], 0.0)
                    break
                nz = L - k + 1
                for mt in range(2):
                    for grp in range(BC // 4):
                        zp = z_ps.tile([128, 4 * nz], f32, tag="zp")
                        zp3 = zp[:].rearrange("p (b t) -> p b t", b=4)
                        n = 0
                        nmm = 3 * (k + 2) + k
                        for s in range(k + 2):
                            for ec in range(3):
                                seg = s * 3 + ec
                                nc.tensor.matmul(
                                    zp3[:, :, :],
                                    cw_we_sb[ki][:ECH[ec],
                                                 seg * FN + mt * 128:
                                                 seg * FN + mt * 128 + 128],
                                    weT3[ec][:ECH[ec], 4 * grp:4 * grp + 4,
                                             s:s + nz],
                                    start=(n == 0), stop=(n == nmm - 1))
                                n += 1
                        for j in range(k):
                            seg = sum(FILTERS[:ki]) + j
                            nc.tensor.matmul(
                                zp3[:, :, :],
                                cw_pos_sb[:2 * P,
                                          seg * FN + mt * 128:
                                          seg * FN + mt * 128 + 128],
                                posT3[:, 4 * grp:4 * grp + 4, j:j + nz],
                                start=(n == 0), stop=(n == nmm - 1))
                            n += 1
                        # subtract t=0 correction for these 4 samples
                        nc.vector.tensor_sub(
                            zp3[:, :, 0:1],
                            zp3[:, :, 0:1],
                            cor_sb[:, (2 * ki + mt) * BC + 4 * grp:
                                   (2 * ki + mt) * BC + 4 * grp + 4]
                            .rearrange("p (b o) -> p b o", o=1))
                        nc.vector.reduce_max(
                            sf[2 * ki + mt][:, 4 * grp:4 * grp + 4],
                            zp3[:, :, :], axis=mybir.AxisListType.X)

            # ---- phase F: sf tanh, g, y ----
            sft = [pp.tile([128, BC], f32, tag=f"sft{i}", name=f"sft{i}")
                   for i in range(6)]
            gp = aux_ps.tile([128, BC], f32, tag="aux")
            for i in range(6):
                nc.scalar.activation(sft[i][:], sf[i][:],
                                     mybir.ActivationFunctionType.Tanh,
                                     bias=cb_sb[:, i:i + 1])
                nc.tensor.matmul(gp[:H2, :], w1t_sb[:, i * H2:(i + 1) * H2],
                                 sft[i][:], start=(i == 0), stop=(i == 5))
            nc.scalar.activation(g_sb[:H2, :], gp[:H2, :],
                                 mybir.ActivationFunctionType.Tanh,
                                 bias=b1_sb[:H2, :1])

            yp = aux_ps.tile([LAB, BC], f32, tag="aux")
            yp3 = yp[:].rearrange("p (o b) -> p o b", o=1)
            n = 0
            for p in range(NSEL):
                for ec in range(3):
                    nc.tensor.matmul(
                        yp3[:, :, :],
                        w2t_sb[:ECH[ec], (3 * p + ec) * LAB:
                               (3 * p + ec + 1) * LAB],
                        lvec[ec][:ECH[ec], :].rearrange(
                            "p (b s) -> p s b", s=NSEL)[:, p:p + 1, :],
                        start=(n == 0), stop=False)
                    n += 1
            nc.tensor.matmul(yp[:], w2t_sb[:H2, 18 * LAB:19 * LAB],
                             g_sb[:H2, :], start=False, stop=True)
            nc.scalar.activation(y_sb[:], yp[:],
                                 mybir.ActivationFunctionType.Identity,
                                 bias=b2_sb[:, :1])
            nc.sync.dma_start(y_d[:], y_sb[:])

    nc.compile()
    return nc


_NC = None
_LAST = None


def kernel(inputs, e1s, e1e, e2s, e2e, p1, p2, emb, pos1, pos2,
           conv_w3, conv_b3, conv_w4, conv_b4, conv_w5, conv_b5,
           W1, b1, W2, b2):
    global _NC
    from concourse.bass_utils import run_bass_kernel_spmd

    shared = _prep_shared(emb, pos1, pos2, conv_w3, conv_b3, conv_w4, conv_b4,
                          conv_w5, conv_b5, W1, b1, W2, b2)
    in_maps = []
    for c in range(NCORES):
        m = dict(shared)
        m.update(_prep_core(c, inputs, e1s, e1e, e2s, e2e, p1, p2))
        in_maps.append(m)

    if _NC is None:
        _NC = _build_nc()

    trace = bool(int(os.environ.get("ATTCNN_TRACE", "0")))
    res = run_bass_kernel_spmd(_NC, in_maps, core_ids=list(range(NCORES)),
                               trace=trace)
    global _LAST
    _LAST = res
    y = np.zeros((B, LAB), np.float32)
    for c in range(NCORES):
        y[c * BC:(c + 1) * BC] = res.results[c]["y"].T
    return y
